# revision 3
# baseline (speedup 1.0000x reference)
"""Multi-head attention (B=8, N=1024, C=768, H=12) on 8 Trainium2 NeuronCores.

Strategy: pure data parallelism over the batch dimension — each of the 8
cores computes full attention for one batch element; weights are
replicated. No collectives needed.

Per-core dataflow (all matmuls expressed as out = lhsT.T @ rhs on the PE):
  1. xT  = transpose(x)                          (PE identity-transpose, 48 blocks)
  2. qkT = w_qkv[:, :1536].T @ xT  (q,k feature-major)   lhsT = w_qkv chunks
     v   = x @ w_qkv[:, 1536:]    (v token-major)        lhsT = xT chunks
  3. per head pair (2 heads share a 128-row qkT chunk → row-tiled K=64 matmuls):
       scoresT[m,n] = k_h @ q_h^T   (lhsT = kT slice, rhs = qT slice)
       expT = exp(scale * scoresT)  (ScalarE, softmax max-subtraction skipped:
                                     |scores*scale| < ~2, exp is safe in fp32)
       U[n, 0:65] += expT[mchunk].T @ [v_h | 1]  (ones column gives the softmax
                                                  denominator in U[:, 64])
       attn_out[n, h*64:(h+1)*64] = U[:, :64] * (1/U[:, 64])
  4. attn_outT = transpose(attn_out); y = attn_outT.T @ w_proj + b
"""

import os
import sys

for _p in ("/opt/trn_rl_repo", "/root/.axon_site/_ro/trn_rl_repo"):
    if os.path.isdir(_p) and _p not in sys.path:
        sys.path.append(_p)

from contextlib import ExitStack

import numpy as np

import concourse.bass as bass
import concourse.tile as tile
from concourse import bacc, mybir
from concourse.bass_utils import run_bass_kernel_spmd
from concourse.masks import make_identity

FP = mybir.dt.float32
N_CORES = 8
T = 1024  # tokens per core (batch element)
C = 768
H = 12
D = 64
SCALE = D ** (-0.5)
TC = T // 128  # 8 token chunks
CCH = C // 128  # 6 channel chunks
NPAIR = H // 2  # 6 head pairs

Exp = mybir.ActivationFunctionType.Exp


def build(n_cores: int = N_CORES):
    nc = bacc.Bacc(
        "TRN2", target_bir_lowering=False, debug=False, num_devices=n_cores
    )
    x = nc.declare_dram_parameter("x", [T, C], FP, isOutput=False)
    w_qkv = nc.declare_dram_parameter("w_qkv", [C, 3 * C], FP, isOutput=False)
    w_proj = nc.declare_dram_parameter("w_proj", [C, C], FP, isOutput=False)
    b_proj = nc.declare_dram_parameter("b_proj", [C], FP, isOutput=False)
    out = nc.declare_dram_parameter("out", [T, C], FP, isOutput=True)

    xa, wqa, wpa, outa = x.ap(), w_qkv.ap(), w_proj.ap(), out.ap()
    ba = b_proj.ap()
    b_bcast_src = bass.AP(tensor=ba.tensor, offset=ba.offset, ap=[[0, 128]] + ba.ap)

    with tile.TileContext(nc) as tc, ExitStack() as ctx:
        # ---- persistent pools (live for the whole kernel) ----
        consts = ctx.enter_context(tc.tile_pool(name="consts", bufs=1))
        qk_pool = ctx.enter_context(tc.tile_pool(name="qk", bufs=12))
        v_pool = ctx.enter_context(tc.tile_pool(name="v65", bufs=TC))
        ao_pool = ctx.enter_context(tc.tile_pool(name="attn_out", bufs=TC))
        wp_pool = ctx.enter_context(tc.tile_pool(name="wp", bufs=CCH))
        y_pool = ctx.enter_context(tc.tile_pool(name="y", bufs=2))
        r_pool = ctx.enter_context(tc.tile_pool(name="r", bufs=4))

        identity = consts.tile([128, 128], FP)
        make_identity(nc, identity)
        b_bcast = consts.tile([128, C], FP)
        nc.sync.dma_start(b_bcast[:], b_bcast_src)

        wp = []
        for c in range(CCH):
            w = wp_pool.tile([128, C], FP, tag="wp")
            nc.sync.dma_start(w[:], wpa[c * 128 : (c + 1) * 128, :])
            wp.append(w)

        v65 = [v_pool.tile([128, H, D + 1], FP, tag="v65", name="v65") for _ in range(TC)]
        attn_out = [ao_pool.tile([128, C], FP, tag="ao", name="ao") for _ in range(TC)]
        qkT = [qk_pool.tile([128, T], FP, tag="qk", name="qk") for _ in range(12)]

        # ================= phase 1: xT, qkT, v =================
        with (
            tc.tile_pool(name="xstage", bufs=2) as xs_pool,
            tc.tile_pool(name="xT", bufs=CCH) as xT_pool,
            tc.tile_pool(name="mm1", bufs=4, space="PSUM") as mm1,
            tc.tile_pool(name="tp1", bufs=2, space="PSUM") as tp1,
        ):
            xT = [xT_pool.tile([128, T], FP, tag="xT", name="xT") for _ in range(CCH)]
            for t in range(TC):
                xs = xs_pool.tile([128, C], FP, tag="xs")
                nc.sync.dma_start(xs[:], xa[t * 128 : (t + 1) * 128, :])
                for c in range(CCH):
                    ps = tp1.tile([128, 128], FP, tag="tp")
                    nc.tensor.transpose(
                        ps[:], xs[:, c * 128 : (c + 1) * 128], identity[:]
                    )
                    nc.any.tensor_copy(xT[c][:, t * 128 : (t + 1) * 128], ps[:])

            # family 1: qT/kT feature-major (qkT rows 0:1536)
            with tc.tile_pool(name="wq1", bufs=CCH) as wq1_pool:
                wq1 = []
                for c in range(CCH):
                    w = wq1_pool.tile([128, 2 * C], FP, tag="wq1")
                    nc.sync.dma_start(w[:], wqa[c * 128 : (c + 1) * 128, 0 : 2 * C])
                    wq1.append(w)
                for j in range(12):
                    for nh in range(2):
                        ps = mm1.tile([128, 512], FP, tag="mm")
                        for c in range(CCH):
                            nc.tensor.matmul(
                                ps[:],
                                wq1[c][:, j * 128 : (j + 1) * 128],
                                xT[c][:, nh * 512 : (nh + 1) * 512],
                                start=(c == 0),
                                stop=(c == CCH - 1),
                            )
                        nc.any.tensor_copy(
                            qkT[j][:, nh * 512 : (nh + 1) * 512], ps[:]
                        )

            # family 2: v token-major, interleaved with ones column
            with tc.tile_pool(name="wq2", bufs=CCH) as wq2_pool:
                wq2 = []
                for c in range(CCH):
                    w = wq2_pool.tile([128, C], FP, tag="wq2")
                    nc.sync.dma_start(
                        w[:], wqa[c * 128 : (c + 1) * 128, 2 * C : 3 * C]
                    )
                    wq2.append(w)
                for t in range(TC):
                    nc.vector.memset(v65[t][:], 1.0)
                    for nh in range(2):
                        ps = mm1.tile([128, 384], FP, tag="mm")
                        for c in range(CCH):
                            nc.tensor.matmul(
                                ps[:],
                                xT[c][:, t * 128 : (t + 1) * 128],
                                wq2[c][:, nh * 384 : (nh + 1) * 384],
                                start=(c == 0),
                                stop=(c == CCH - 1),
                            )
                        nc.any.tensor_copy(
                            v65[t][:, nh * 6 : (nh + 1) * 6, 0:D],
                            ps.rearrange("p (g d) -> p g d", g=6),
                        )

        # ================= phase 2: attention =================
        with (
            tc.tile_pool(name="expT", bufs=2) as exp_pool,
            tc.tile_pool(name="sc", bufs=3, space="PSUM") as sc_psum,
            tc.tile_pool(name="u", bufs=2, space="PSUM") as u_psum,
        ):
            for p in range(NPAIR):
                eAB = [
                    exp_pool.tile([128, TC, T], FP, tag="expT", name="expT") for _ in range(2)
                ]
                for j in range(TC):  # key-token chunks (m)
                    psAB = [
                        sc_psum.tile([128, T], FP, tag="sc", name="sc") for _ in range(2)
                    ]
                    for half in range(2):
                        base = 64 * half
                        for nh in range(2):
                            nc.tensor.matmul(
                                psAB[half][:, nh * 512 : (nh + 1) * 512],
                                qkT[6 + p][
                                    base : base + 64, j * 128 : (j + 1) * 128
                                ],
                                qkT[p][base : base + 64, nh * 512 : (nh + 1) * 512],
                                start=True,
                                stop=True,
                            )
                        nc.scalar.activation(
                            eAB[half][:, j, :], psAB[half][:], Exp, scale=SCALE
                        )
                for half in range(2):
                    h = 2 * p + half
                    e = eAB[half]
                    for i in range(TC):  # query-token chunks (n)
                        us = u_psum.tile([128, D + 1], FP, tag="u")
                        for j in range(TC):
                            nc.tensor.matmul(
                                us[:],
                                e[:, j, i * 128 : (i + 1) * 128],
                                v65[j][:, h, :],
                                start=(j == 0),
                                stop=(j == TC - 1),
                            )
                        r = r_pool.tile([128, 1], FP, tag="r")
                        nc.vector.reciprocal(r[:], us[:, D : D + 1])
                        nc.vector.tensor_scalar_mul(
                            attn_out[i][:, h * D : (h + 1) * D], us[:, 0:D], r[:]
                        )

        # ================= phase 3: attn_outT, proj =================
        with (
            tc.tile_pool(name="aoT", bufs=CCH) as aoT_pool,
            tc.tile_pool(name="tp2", bufs=2, space="PSUM") as tp2,
            tc.tile_pool(name="pj", bufs=4, space="PSUM") as pj,
        ):
            aoT = [aoT_pool.tile([128, T], FP, tag="aoT", name="aoT") for _ in range(CCH)]
            for t in range(TC):
                for c in range(CCH):
                    ps = tp2.tile([128, 128], FP, tag="tp")
                    nc.tensor.transpose(
                        ps[:], attn_out[t][:, c * 128 : (c + 1) * 128], identity[:]
                    )
                    nc.any.tensor_copy(aoT[c][:, t * 128 : (t + 1) * 128], ps[:])
            for t in range(TC):
                y = y_pool.tile([128, C], FP, tag="y")
                for nh in range(2):
                    ps = pj.tile([128, 384], FP, tag="pj")
                    for c in range(CCH):
                        nc.tensor.matmul(
                            ps[:],
                            aoT[c][:, t * 128 : (t + 1) * 128],
                            wp[c][:, nh * 384 : (nh + 1) * 384],
                            start=(c == 0),
                            stop=(c == CCH - 1),
                        )
                    nc.vector.tensor_add(
                        y[:, nh * 384 : (nh + 1) * 384],
                        ps[:],
                        b_bcast[:, nh * 384 : (nh + 1) * 384],
                    )
                nc.sync.dma_start(outa[t * 128 : (t + 1) * 128, :], y[:])

    nc.finalize()
    return nc


_NC_CACHE = {}


def _get_nc():
    if "nc" not in _NC_CACHE:
        _NC_CACHE["nc"] = build()
    return _NC_CACHE["nc"]


def kernel(x, w_qkv, w_proj, b_proj):
    """Full inputs in, full output out. Shards batch across 8 NeuronCores."""
    assert x.shape == (N_CORES, T, C), x.shape
    nc = _get_nc()
    in_maps = [
        {
            "x": np.ascontiguousarray(x[i], dtype=np.float32),
            "w_qkv": np.ascontiguousarray(w_qkv, dtype=np.float32),
            "w_proj": np.ascontiguousarray(w_proj, dtype=np.float32),
            "b_proj": np.ascontiguousarray(b_proj, dtype=np.float32),
        }
        for i in range(N_CORES)
    ]
    res = run_bass_kernel_spmd(nc, in_maps, list(range(N_CORES)))
    return np.stack([res.results[i]["out"] for i in range(N_CORES)], axis=0)


# revision 6
# speedup vs baseline: 1.0828x; 1.0828x over previous
"""Multi-head attention (B=8, N=1024, C=768, H=12) on 8 Trainium2 NeuronCores.

Strategy: pure data parallelism over the batch dimension — each of the 8
cores computes full attention for one batch element; weights are
replicated. No collectives needed.

Per-core dataflow (all matmuls expressed as out = lhsT.T @ rhs on the PE):
  1. xT  = transpose(x)                          (PE identity-transpose, 48 blocks)
  2. qkT = w_qkv[:, :1536].T @ xT  (q,k feature-major)   lhsT = w_qkv chunks
     v   = x @ w_qkv[:, 1536:]    (v token-major)        lhsT = xT chunks
  3. per head pair (2 heads share a 128-row qkT chunk → row-tiled K=64 matmuls):
       scoresT[m,n] = k_h @ q_h^T   (lhsT = kT slice, rhs = qT slice)
       expT = exp(scale * scoresT)  (ScalarE, softmax max-subtraction skipped:
                                     |scores*scale| < ~2, exp is safe in fp32)
       U[n, 0:65] += expT[mchunk].T @ [v_h | 1]  (ones column gives the softmax
                                                  denominator in U[:, 64])
       attn_out[n, h*64:(h+1)*64] = U[:, :64] * (1/U[:, 64])
  4. attn_outT = transpose(attn_out); y = attn_outT.T @ w_proj + b
"""

import os
import sys

for _p in ("/opt/trn_rl_repo", "/root/.axon_site/_ro/trn_rl_repo"):
    if os.path.isdir(_p) and _p not in sys.path:
        sys.path.append(_p)

from contextlib import ExitStack

import numpy as np

import concourse.bass as bass
import concourse.tile as tile
from concourse import bacc, mybir
from concourse.bass_utils import run_bass_kernel_spmd
from concourse.masks import make_identity

FP = mybir.dt.float32
F32R = mybir.dt.float32r
N_CORES = 8
T = 1024  # tokens per core (batch element)
C = 768
H = 12
D = 64
SCALE = D ** (-0.5)
TC = T // 128  # 8 token chunks
CCH = C // 128  # 6 channel chunks
NPAIR = H // 2  # 6 head pairs

Exp = mybir.ActivationFunctionType.Exp


def build(n_cores: int = N_CORES, fast: bool = True):
    # fast=True: run the large matmuls (qkv / scores / proj, all N>=256) with
    # operands tagged float32r — the PE's full-rate fp32 mode (1 cycle/row vs
    # 4 for plain fp32). Storage is IEEE fp32 either way; only the matmul
    # interpretation changes.
    MMDT = F32R if fast else FP
    nc = bacc.Bacc(
        "TRN2", target_bir_lowering=False, debug=False, num_devices=n_cores
    )
    wdma = nc.gpsimd.dma_start if fast else nc.sync.dma_start
    x = nc.declare_dram_parameter("x", [T, C], FP, isOutput=False)
    w_qkv = nc.declare_dram_parameter("w_qkv", [C, 3 * C], FP, isOutput=False)
    w_proj = nc.declare_dram_parameter("w_proj", [C, C], FP, isOutput=False)
    b_proj = nc.declare_dram_parameter("b_proj", [C], FP, isOutput=False)
    out = nc.declare_dram_parameter("out", [T, C], FP, isOutput=True)

    xa, wqa, wpa, outa = x.ap(), w_qkv.ap(), w_proj.ap(), out.ap()
    ba = b_proj.ap()
    b_bcast_src = bass.AP(tensor=ba.tensor, offset=ba.offset, ap=[[0, 128]] + ba.ap)

    with tile.TileContext(nc) as tc, ExitStack() as ctx:
        # ---- persistent pools (live for the whole kernel) ----
        consts = ctx.enter_context(tc.tile_pool(name="consts", bufs=1))
        qk_pool = ctx.enter_context(tc.tile_pool(name="qk", bufs=12))
        v_pool = ctx.enter_context(tc.tile_pool(name="v65", bufs=TC))
        ao_pool = ctx.enter_context(tc.tile_pool(name="attn_out", bufs=TC))
        wp_pool = ctx.enter_context(tc.tile_pool(name="wp", bufs=CCH))
        y_pool = ctx.enter_context(tc.tile_pool(name="y", bufs=2))
        r_pool = ctx.enter_context(tc.tile_pool(name="r", bufs=4))

        identity = consts.tile([128, 128], FP)
        make_identity(nc, identity)
        b_bcast = consts.tile([128, C], FP)
        nc.sync.dma_start(b_bcast[:], b_bcast_src)

        wp = []
        for c in range(CCH):
            w = wp_pool.tile([128, C], MMDT, tag="wp")
            wdma(w[:], wpa[c * 128 : (c + 1) * 128, :])
            wp.append(w)

        v65 = [v_pool.tile([128, H, D + 1], FP, tag="v65", name="v65") for _ in range(TC)]
        attn_out = [ao_pool.tile([128, C], FP, tag="ao", name="ao") for _ in range(TC)]
        qkT = [qk_pool.tile([128, T], MMDT, tag="qk", name="qk") for _ in range(12)]

        # ================= phase 1: xT, qkT, v =================
        with (
            tc.tile_pool(name="xstage", bufs=2) as xs_pool,
            tc.tile_pool(name="xT", bufs=CCH) as xT_pool,
            tc.tile_pool(name="mm1", bufs=4, space="PSUM") as mm1,
            tc.tile_pool(name="tp1", bufs=2, space="PSUM") as tp1,
        ):
            xT = [xT_pool.tile([128, T], MMDT, tag="xT", name="xT") for _ in range(CCH)]
            for t in range(TC):
                xs = xs_pool.tile([128, C], FP, tag="xs")
                nc.sync.dma_start(xs[:], xa[t * 128 : (t + 1) * 128, :])
                for c in range(CCH):
                    ps = tp1.tile([128, 128], FP, tag="tp")
                    nc.tensor.transpose(
                        ps[:], xs[:, c * 128 : (c + 1) * 128], identity[:]
                    )
                    nc.any.tensor_copy(xT[c][:, t * 128 : (t + 1) * 128], ps[:])

            # family 1: qT/kT feature-major (qkT rows 0:1536)
            with tc.tile_pool(name="wq1", bufs=CCH) as wq1_pool:
                wq1 = []
                for c in range(CCH):
                    w = wq1_pool.tile([128, 2 * C], MMDT, tag="wq1")
                    wdma(w[:], wqa[c * 128 : (c + 1) * 128, 0 : 2 * C])
                    wq1.append(w)
                for j in range(12):
                    for nh in range(2):
                        ps = mm1.tile([128, 512], FP, tag="mm")
                        for c in range(CCH):
                            nc.tensor.matmul(
                                ps[:],
                                wq1[c][:, j * 128 : (j + 1) * 128],
                                xT[c][:, nh * 512 : (nh + 1) * 512],
                                start=(c == 0),
                                stop=(c == CCH - 1),
                            )
                        nc.any.tensor_copy(
                            qkT[j][:, nh * 512 : (nh + 1) * 512], ps[:]
                        )

            # family 2: v token-major, interleaved with ones column
            with tc.tile_pool(name="wq2", bufs=CCH) as wq2_pool:
                wq2 = []
                for c in range(CCH):
                    w = wq2_pool.tile([128, C], MMDT, tag="wq2")
                    wdma(
                        w[:], wqa[c * 128 : (c + 1) * 128, 2 * C : 3 * C]
                    )
                    wq2.append(w)
                for t in range(TC):
                    nc.vector.memset(v65[t][:], 1.0)
                    for nh in range(2):
                        ps = mm1.tile([128, 384], FP, tag="mm")
                        for c in range(CCH):
                            nc.tensor.matmul(
                                ps[:],
                                xT[c][:, t * 128 : (t + 1) * 128],
                                wq2[c][:, nh * 384 : (nh + 1) * 384],
                                start=(c == 0),
                                stop=(c == CCH - 1),
                            )
                        nc.any.tensor_copy(
                            v65[t][:, nh * 6 : (nh + 1) * 6, 0:D],
                            ps.rearrange("p (g d) -> p g d", g=6),
                        )

        # ================= phase 2: attention =================
        with (
            tc.tile_pool(name="expT", bufs=2) as exp_pool,
            tc.tile_pool(name="sc", bufs=3, space="PSUM") as sc_psum,
            tc.tile_pool(name="u", bufs=2, space="PSUM") as u_psum,
        ):
            for p in range(NPAIR):
                eAB = [
                    exp_pool.tile([128, TC, T], FP, tag="expT", name="expT") for _ in range(2)
                ]
                for j in range(TC):  # key-token chunks (m)
                    psAB = [
                        sc_psum.tile([128, T], FP, tag="sc", name="sc") for _ in range(2)
                    ]
                    for half in range(2):
                        base = 64 * half
                        for nh in range(2):
                            nc.tensor.matmul(
                                psAB[half][:, nh * 512 : (nh + 1) * 512],
                                qkT[6 + p][
                                    base : base + 64, j * 128 : (j + 1) * 128
                                ],
                                qkT[p][base : base + 64, nh * 512 : (nh + 1) * 512],
                                start=True,
                                stop=True,
                            )
                        nc.scalar.activation(
                            eAB[half][:, j, :], psAB[half][:], Exp, scale=SCALE
                        )
                for half in range(2):
                    h = 2 * p + half
                    e = eAB[half]
                    for i in range(TC):  # query-token chunks (n)
                        us = u_psum.tile([128, D + 1], FP, tag="u")
                        for j in range(TC):
                            nc.tensor.matmul(
                                us[:],
                                e[:, j, i * 128 : (i + 1) * 128],
                                v65[j][:, h, :],
                                start=(j == 0),
                                stop=(j == TC - 1),
                            )
                        r = r_pool.tile([128, 1], FP, tag="r")
                        nc.vector.reciprocal(r[:], us[:, D : D + 1])
                        nc.vector.tensor_scalar_mul(
                            attn_out[i][:, h * D : (h + 1) * D], us[:, 0:D], r[:]
                        )

        # ================= phase 3: attn_outT, proj =================
        with (
            tc.tile_pool(name="aoT", bufs=CCH) as aoT_pool,
            tc.tile_pool(name="tp2", bufs=2, space="PSUM") as tp2,
            tc.tile_pool(name="pj", bufs=4, space="PSUM") as pj,
        ):
            aoT = [aoT_pool.tile([128, T], MMDT, tag="aoT", name="aoT") for _ in range(CCH)]
            for t in range(TC):
                for c in range(CCH):
                    ps = tp2.tile([128, 128], FP, tag="tp")
                    nc.tensor.transpose(
                        ps[:], attn_out[t][:, c * 128 : (c + 1) * 128], identity[:]
                    )
                    nc.any.tensor_copy(aoT[c][:, t * 128 : (t + 1) * 128], ps[:])
            for t in range(TC):
                y = y_pool.tile([128, C], FP, tag="y")
                for nh in range(2):
                    ps = pj.tile([128, 384], FP, tag="pj")
                    for c in range(CCH):
                        nc.tensor.matmul(
                            ps[:],
                            aoT[c][:, t * 128 : (t + 1) * 128],
                            wp[c][:, nh * 384 : (nh + 1) * 384],
                            start=(c == 0),
                            stop=(c == CCH - 1),
                        )
                    nc.vector.tensor_add(
                        y[:, nh * 384 : (nh + 1) * 384],
                        ps[:],
                        b_bcast[:, nh * 384 : (nh + 1) * 384],
                    )
                nc.sync.dma_start(outa[t * 128 : (t + 1) * 128, :], y[:])

    nc.finalize()
    return nc


_NC_CACHE = {}


def _get_nc():
    if "nc" not in _NC_CACHE:
        import os

        fast = os.environ.get("KERNEL_FAST", "1") == "1"
        _NC_CACHE["nc"] = build(fast=fast)
    return _NC_CACHE["nc"]


def kernel(x, w_qkv, w_proj, b_proj):
    """Full inputs in, full output out. Shards batch across 8 NeuronCores."""
    assert x.shape == (N_CORES, T, C), x.shape
    nc = _get_nc()
    in_maps = [
        {
            "x": np.ascontiguousarray(x[i], dtype=np.float32),
            "w_qkv": np.ascontiguousarray(w_qkv, dtype=np.float32),
            "w_proj": np.ascontiguousarray(w_proj, dtype=np.float32),
            "b_proj": np.ascontiguousarray(b_proj, dtype=np.float32),
        }
        for i in range(N_CORES)
    ]
    res = run_bass_kernel_spmd(nc, in_maps, list(range(N_CORES)))
    return np.stack([res.results[i]["out"] for i in range(N_CORES)], axis=0)


# revision 10
# speedup vs baseline: 1.5008x; 1.3861x over previous
"""Multi-head attention (B=8, N=1024, C=768, H=12) on 8 Trainium2 NeuronCores.

Strategy: pure data parallelism over the batch dimension — each of the 8
cores computes full attention for one batch element; weights are
replicated. No collectives needed.

Per-core dataflow (all matmuls expressed as out = lhsT.T @ rhs on the PE):
  1. xT  = transpose(x)                          (PE identity-transpose, 48 blocks)
  2. qkT = w_qkv[:, :1536].T @ xT  (q,k feature-major)   lhsT = w_qkv chunks
     v   = x @ w_qkv[:, 1536:]    (v token-major)        lhsT = xT chunks
  3. per head pair (2 heads share a 128-row qkT chunk → row-tiled K=64 matmuls):
       scoresT[m,n] = k_h @ q_h^T   (lhsT = kT slice, rhs = qT slice)
       expT = exp(scale * scoresT)  (ScalarE, softmax max-subtraction skipped:
                                     |scores*scale| < ~2, exp is safe in fp32)
       U[n, 0:65] += expT[mchunk].T @ [v_h | 1]  (ones column gives the softmax
                                                  denominator in U[:, 64])
       attn_out[n, h*64:(h+1)*64] = U[:, :64] * (1/U[:, 64])
  4. attn_outT = transpose(attn_out); y = attn_outT.T @ w_proj + b
"""

import os
import sys

for _p in ("/opt/trn_rl_repo", "/root/.axon_site/_ro/trn_rl_repo"):
    if os.path.isdir(_p) and _p not in sys.path:
        sys.path.append(_p)

from contextlib import ExitStack

import numpy as np

import concourse.bass as bass
import concourse.tile as tile
from concourse import bacc, mybir
from concourse.bass_utils import run_bass_kernel_spmd
from concourse.masks import make_identity

FP = mybir.dt.float32
F32R = mybir.dt.float32r
N_CORES = 8
T = 1024  # tokens per core (batch element)
C = 768
H = 12
D = 64
SCALE = D ** (-0.5)
TC = T // 128  # 8 token chunks
CCH = C // 128  # 6 channel chunks
NPAIR = H // 2  # 6 head pairs

Exp = mybir.ActivationFunctionType.Exp


def build(n_cores: int = N_CORES, fast: bool = True):
    # fast=True: run the large matmuls (qkv / scores / proj, all N>=256) with
    # operands tagged float32r — the PE's full-rate fp32 mode (1 cycle/row vs
    # 4 for plain fp32). Storage is IEEE fp32 either way; only the matmul
    # interpretation changes.
    MMDT = F32R if fast else FP
    nc = bacc.Bacc(
        "TRN2", target_bir_lowering=False, debug=False, num_devices=n_cores
    )
    wdma = nc.gpsimd.dma_start if fast else nc.sync.dma_start
    x = nc.declare_dram_parameter("x", [T, C], FP, isOutput=False)
    w_qkv = nc.declare_dram_parameter("w_qkv", [C, 3 * C], FP, isOutput=False)
    w_proj = nc.declare_dram_parameter("w_proj", [C, C], FP, isOutput=False)
    b_proj = nc.declare_dram_parameter("b_proj", [C], FP, isOutput=False)
    out = nc.declare_dram_parameter("out", [T, C], FP, isOutput=True)

    xa, wqa, wpa, outa = x.ap(), w_qkv.ap(), w_proj.ap(), out.ap()
    ba = b_proj.ap()
    b_bcast_src = bass.AP(tensor=ba.tensor, offset=ba.offset, ap=[[0, 128]] + ba.ap)

    with tile.TileContext(nc) as tc, ExitStack() as ctx:
        # ---- persistent pools (live for the whole kernel) ----
        consts = ctx.enter_context(tc.tile_pool(name="consts", bufs=1))
        qk_pool = ctx.enter_context(tc.tile_pool(name="qk", bufs=12))
        v_pool = ctx.enter_context(tc.tile_pool(name="v65", bufs=TC))
        ao_pool = ctx.enter_context(tc.tile_pool(name="attn_out", bufs=TC))
        wp_pool = ctx.enter_context(tc.tile_pool(name="wp", bufs=CCH))
        y_pool = ctx.enter_context(tc.tile_pool(name="y", bufs=2))
        r_pool = ctx.enter_context(tc.tile_pool(name="r", bufs=4))

        identity = consts.tile([128, 128], FP)
        make_identity(nc, identity)
        ones_h = consts.tile([128, 2 * H], FP)
        nc.vector.memset(ones_h[:], 1.0)
        b_bcast = consts.tile([128, C], FP)
        nc.sync.dma_start(b_bcast[:], b_bcast_src)

        wp = []
        for c in range(CCH):
            w = wp_pool.tile([128, C], MMDT, tag="wp")
            wdma(w[:], wpa[c * 128 : (c + 1) * 128, :])
            wp.append(w)

        v65 = [v_pool.tile([128, H, D + 2], MMDT, tag="v65", name="v65") for _ in range(TC)]
        attn_out = [ao_pool.tile([128, C], FP, tag="ao", name="ao") for _ in range(TC)]
        qkT = [qk_pool.tile([128, T], MMDT, tag="qk", name="qk") for _ in range(12)]

        # ================= phase 1: xT, qkT, v =================
        with (
            tc.tile_pool(name="xstage", bufs=2) as xs_pool,
            tc.tile_pool(name="xT", bufs=CCH) as xT_pool,
            tc.tile_pool(name="mm1", bufs=4, space="PSUM") as mm1,
            tc.tile_pool(name="tp1", bufs=2, space="PSUM") as tp1,
        ):
            xT = [xT_pool.tile([128, T], MMDT, tag="xT", name="xT") for _ in range(CCH)]
            for t in range(TC):
                xs = xs_pool.tile([128, C], FP, tag="xs")
                nc.sync.dma_start(xs[:], xa[t * 128 : (t + 1) * 128, :])
                for c in range(CCH):
                    ps = tp1.tile([128, 128], FP, tag="tp")
                    nc.tensor.transpose(
                        ps[:], xs[:, c * 128 : (c + 1) * 128], identity[:]
                    )
                    nc.any.tensor_copy(xT[c][:, t * 128 : (t + 1) * 128], ps[:])

            # family 1: qT/kT feature-major (qkT rows 0:1536)
            with tc.tile_pool(name="wq1", bufs=CCH) as wq1_pool:
                wq1 = []
                for c in range(CCH):
                    w = wq1_pool.tile([128, 2 * C], MMDT, tag="wq1")
                    wdma(w[:], wqa[c * 128 : (c + 1) * 128, 0 : 2 * C])
                    wq1.append(w)
                for j in range(12):
                    for nh in range(2):
                        ps = mm1.tile([128, 512], FP, tag="mm")
                        for c in range(CCH):
                            nc.tensor.matmul(
                                ps[:],
                                wq1[c][:, j * 128 : (j + 1) * 128],
                                xT[c][:, nh * 512 : (nh + 1) * 512],
                                start=(c == 0),
                                stop=(c == CCH - 1),
                            )
                        nc.any.tensor_copy(
                            qkT[j][:, nh * 512 : (nh + 1) * 512], ps[:]
                        )

            # family 2: v token-major, interleaved with ones column
            with tc.tile_pool(name="wq2", bufs=CCH) as wq2_pool:
                wq2 = []
                for c in range(CCH):
                    w = wq2_pool.tile([128, C], MMDT, tag="wq2")
                    wdma(
                        w[:], wqa[c * 128 : (c + 1) * 128, 2 * C : 3 * C]
                    )
                    wq2.append(w)
                for t in range(TC):
                    if fast:
                        # memset can't emit float32r; write the ones column
                        # via a casting DMA (1.0 is exact in any rounding)
                        nc.gpsimd.dma_start(
                            out=v65[t][:, :, D : D + 2],
                            in_=ones_h[:].rearrange("p (h o) -> p h o", o=2),
                        )
                    else:
                        nc.vector.memset(v65[t][:], 1.0)
                    for nh in range(2):
                        ps = mm1.tile([128, 384], FP, tag="mm")
                        for c in range(CCH):
                            nc.tensor.matmul(
                                ps[:],
                                xT[c][:, t * 128 : (t + 1) * 128],
                                wq2[c][:, nh * 384 : (nh + 1) * 384],
                                start=(c == 0),
                                stop=(c == CCH - 1),
                            )
                        nc.any.tensor_copy(
                            v65[t][:, nh * 6 : (nh + 1) * 6, 0:D],
                            ps.rearrange("p (g d) -> p g d", g=6),
                        )

        # ================= phase 2: attention =================
        with (
            tc.tile_pool(name="expT", bufs=2) as exp_pool,
            tc.tile_pool(name="sc", bufs=3, space="PSUM") as sc_psum,
            tc.tile_pool(name="u", bufs=2, space="PSUM") as u_psum,
        ):
            for p in range(NPAIR):
                eAB = [
                    exp_pool.tile([128, TC, T], MMDT, tag="expT", name="expT") for _ in range(2)
                ]
                for j in range(TC):  # key-token chunks (m)
                    psAB = [
                        sc_psum.tile([128, T], FP, tag="sc", name="sc") for _ in range(2)
                    ]
                    for half in range(2):
                        base = 64 * half
                        for nh in range(2):
                            nc.tensor.matmul(
                                psAB[half][:, nh * 512 : (nh + 1) * 512],
                                qkT[6 + p][
                                    base : base + 64, j * 128 : (j + 1) * 128
                                ],
                                qkT[p][base : base + 64, nh * 512 : (nh + 1) * 512],
                                start=True,
                                stop=True,
                            )
                        nc.scalar.activation(
                            eAB[half][:, j, :], psAB[half][:], Exp, scale=SCALE
                        )
                for half in range(2):
                    h = 2 * p + half
                    e = eAB[half]
                    for i in range(TC):  # query-token chunks (n)
                        us = u_psum.tile([128, D + 2], FP, tag="u")
                        for j in range(TC):
                            nc.tensor.matmul(
                                us[:],
                                e[:, j, i * 128 : (i + 1) * 128],
                                v65[j][:, h, :],
                                start=(j == 0),
                                stop=(j == TC - 1),
                            )
                        r = r_pool.tile([128, 1], FP, tag="r")
                        nc.vector.reciprocal(r[:], us[:, D : D + 1])
                        nc.vector.tensor_scalar_mul(
                            attn_out[i][:, h * D : (h + 1) * D], us[:, 0:D], r[:]
                        )

        # ================= phase 3: attn_outT, proj =================
        with (
            tc.tile_pool(name="aoT", bufs=CCH) as aoT_pool,
            tc.tile_pool(name="tp2", bufs=2, space="PSUM") as tp2,
            tc.tile_pool(name="pj", bufs=4, space="PSUM") as pj,
        ):
            aoT = [aoT_pool.tile([128, T], MMDT, tag="aoT", name="aoT") for _ in range(CCH)]
            for t in range(TC):
                for c in range(CCH):
                    ps = tp2.tile([128, 128], FP, tag="tp")
                    nc.tensor.transpose(
                        ps[:], attn_out[t][:, c * 128 : (c + 1) * 128], identity[:]
                    )
                    nc.any.tensor_copy(aoT[c][:, t * 128 : (t + 1) * 128], ps[:])
            for t in range(TC):
                y = y_pool.tile([128, C], FP, tag="y")
                for nh in range(2):
                    ps = pj.tile([128, 384], FP, tag="pj")
                    for c in range(CCH):
                        nc.tensor.matmul(
                            ps[:],
                            aoT[c][:, t * 128 : (t + 1) * 128],
                            wp[c][:, nh * 384 : (nh + 1) * 384],
                            start=(c == 0),
                            stop=(c == CCH - 1),
                        )
                    nc.vector.tensor_add(
                        y[:, nh * 384 : (nh + 1) * 384],
                        ps[:],
                        b_bcast[:, nh * 384 : (nh + 1) * 384],
                    )
                nc.sync.dma_start(outa[t * 128 : (t + 1) * 128, :], y[:])

    nc.finalize()
    return nc


_NC_CACHE = {}


def _get_nc():
    if "nc" not in _NC_CACHE:
        import os

        fast = os.environ.get("KERNEL_FAST", "1") == "1"
        _NC_CACHE["nc"] = build(fast=fast)
    return _NC_CACHE["nc"]


def kernel(x, w_qkv, w_proj, b_proj):
    """Full inputs in, full output out. Shards batch across 8 NeuronCores."""
    assert x.shape == (N_CORES, T, C), x.shape
    nc = _get_nc()
    in_maps = [
        {
            "x": np.ascontiguousarray(x[i], dtype=np.float32),
            "w_qkv": np.ascontiguousarray(w_qkv, dtype=np.float32),
            "w_proj": np.ascontiguousarray(w_proj, dtype=np.float32),
            "b_proj": np.ascontiguousarray(b_proj, dtype=np.float32),
        }
        for i in range(N_CORES)
    ]
    res = run_bass_kernel_spmd(nc, in_maps, list(range(N_CORES)))
    return np.stack([res.results[i]["out"] for i in range(N_CORES)], axis=0)


# revision 11
# speedup vs baseline: 1.9021x; 1.2673x over previous
"""Multi-head attention (B=8, N=1024, C=768, H=12) on 8 Trainium2 NeuronCores.

Strategy: pure data parallelism over the batch dimension — each of the 8
cores computes full attention for one batch element; weights are
replicated. No collectives needed.

Per-core dataflow (all matmuls expressed as out = lhsT.T @ rhs on the PE):
  1. xT  = transpose(x)                          (PE identity-transpose, 48 blocks)
  2. qkT = w_qkv[:, :1536].T @ xT  (q,k feature-major)   lhsT = w_qkv chunks
     v   = x @ w_qkv[:, 1536:]    (v token-major)        lhsT = xT chunks
  3. per head pair (2 heads share a 128-row qkT chunk → row-tiled K=64 matmuls):
       scoresT[m,n] = k_h @ q_h^T   (lhsT = kT slice, rhs = qT slice)
       expT = exp(scale * scoresT)  (ScalarE, softmax max-subtraction skipped:
                                     |scores*scale| < ~2, exp is safe in fp32)
       U[n, 0:65] += expT[mchunk].T @ [v_h | 1]  (ones column gives the softmax
                                                  denominator in U[:, 64])
       attn_out[n, h*64:(h+1)*64] = U[:, :64] * (1/U[:, 64])
  4. attn_outT = transpose(attn_out); y = attn_outT.T @ w_proj + b
"""

import os
import sys

for _p in ("/opt/trn_rl_repo", "/root/.axon_site/_ro/trn_rl_repo"):
    if os.path.isdir(_p) and _p not in sys.path:
        sys.path.append(_p)

from contextlib import ExitStack

import numpy as np

import concourse.bass as bass
import concourse.tile as tile
from concourse import bacc, mybir
from concourse.bass_utils import run_bass_kernel_spmd
from concourse.masks import make_identity

FP = mybir.dt.float32
F32R = mybir.dt.float32r
N_CORES = 8
T = 1024  # tokens per core (batch element)
C = 768
H = 12
D = 64
SCALE = D ** (-0.5)
TC = T // 128  # 8 token chunks
CCH = C // 128  # 6 channel chunks
NPAIR = H // 2  # 6 head pairs

Exp = mybir.ActivationFunctionType.Exp


def build(n_cores: int = N_CORES, fast: bool = True):
    # fast=True: run the large matmuls (qkv / scores / proj, all N>=256) with
    # operands tagged float32r — the PE's full-rate fp32 mode (1 cycle/row vs
    # 4 for plain fp32). Storage is IEEE fp32 either way; only the matmul
    # interpretation changes.
    MMDT = F32R if fast else FP
    nc = bacc.Bacc(
        "TRN2", target_bir_lowering=False, debug=False, num_devices=n_cores
    )
    wdma = nc.gpsimd.dma_start if fast else nc.sync.dma_start
    x = nc.declare_dram_parameter("x", [T, C], FP, isOutput=False)
    w_qkv = nc.declare_dram_parameter("w_qkv", [C, 3 * C], FP, isOutput=False)
    w_proj = nc.declare_dram_parameter("w_proj", [C, C], FP, isOutput=False)
    b_proj = nc.declare_dram_parameter("b_proj", [C], FP, isOutput=False)
    out = nc.declare_dram_parameter("out", [T, C], FP, isOutput=True)

    xa, wqa, wpa, outa = x.ap(), w_qkv.ap(), w_proj.ap(), out.ap()
    ba = b_proj.ap()
    b_bcast_src = bass.AP(tensor=ba.tensor, offset=ba.offset, ap=[[0, 128]] + ba.ap)

    with tile.TileContext(nc) as tc, ExitStack() as ctx:
        # ---- persistent pools (live for the whole kernel) ----
        consts = ctx.enter_context(tc.tile_pool(name="consts", bufs=1))
        qk_pool = ctx.enter_context(tc.tile_pool(name="qk", bufs=12))
        v_pool = ctx.enter_context(tc.tile_pool(name="v65", bufs=TC))
        ao_pool = ctx.enter_context(tc.tile_pool(name="attn_out", bufs=TC))
        wp_pool = ctx.enter_context(tc.tile_pool(name="wp", bufs=CCH))
        y_pool = ctx.enter_context(tc.tile_pool(name="y", bufs=2))
        r_pool = ctx.enter_context(tc.tile_pool(name="r", bufs=4))

        identity = consts.tile([128, 128], FP)
        make_identity(nc, identity)
        ones_h = consts.tile([128, 2 * H], FP)
        nc.vector.memset(ones_h[:], 1.0)
        b_bcast = consts.tile([128, C], FP)
        nc.sync.dma_start(b_bcast[:], b_bcast_src)

        wp = []
        for c in range(CCH):
            w = wp_pool.tile([128, C], MMDT, tag="wp")
            wdma(w[:], wpa[c * 128 : (c + 1) * 128, :])
            wp.append(w)

        v65 = [v_pool.tile([128, H, D + 2], MMDT, tag="v65", name="v65") for _ in range(TC)]
        attn_out = [ao_pool.tile([128, C], FP, tag="ao", name="ao") for _ in range(TC)]
        qkT = [qk_pool.tile([128, T], MMDT, tag="qk", name="qk") for _ in range(12)]

        # ================= phase 1: xT, qkT, v =================
        with (
            tc.tile_pool(name="xstage", bufs=2) as xs_pool,
            tc.tile_pool(name="xT", bufs=CCH) as xT_pool,
            tc.tile_pool(name="mm1", bufs=4, space="PSUM") as mm1,
            tc.tile_pool(name="tp1", bufs=2, space="PSUM") as tp1,
        ):
            xT = [xT_pool.tile([128, T], MMDT, tag="xT", name="xT") for _ in range(CCH)]
            for t in range(TC):
                xs = xs_pool.tile([128, C], FP, tag="xs")
                nc.sync.dma_start(xs[:], xa[t * 128 : (t + 1) * 128, :])
                for c in range(CCH):
                    ps = tp1.tile([128, 128], FP, tag="tp")
                    nc.tensor.transpose(
                        ps[:], xs[:, c * 128 : (c + 1) * 128], identity[:]
                    )
                    nc.any.tensor_copy(xT[c][:, t * 128 : (t + 1) * 128], ps[:])

            # family 1: qT/kT feature-major (qkT rows 0:1536)
            with tc.tile_pool(name="wq1", bufs=CCH) as wq1_pool:
                wq1 = []
                for c in range(CCH):
                    w = wq1_pool.tile([128, 2 * C], MMDT, tag="wq1")
                    wdma(w[:], wqa[c * 128 : (c + 1) * 128, 0 : 2 * C])
                    wq1.append(w)
                for j in range(12):
                    for nh in range(2):
                        ps = mm1.tile([128, 512], FP, tag="mm")
                        for c in range(CCH):
                            nc.tensor.matmul(
                                ps[:],
                                wq1[c][:, j * 128 : (j + 1) * 128],
                                xT[c][:, nh * 512 : (nh + 1) * 512],
                                start=(c == 0),
                                stop=(c == CCH - 1),
                            )
                        nc.any.tensor_copy(
                            qkT[j][:, nh * 512 : (nh + 1) * 512], ps[:]
                        )

            # family 2: v token-major, interleaved with ones column
            with tc.tile_pool(name="wq2", bufs=CCH) as wq2_pool:
                wq2 = []
                for c in range(CCH):
                    w = wq2_pool.tile([128, C], MMDT, tag="wq2")
                    wdma(
                        w[:], wqa[c * 128 : (c + 1) * 128, 2 * C : 3 * C]
                    )
                    wq2.append(w)
                for t in range(TC):
                    if fast:
                        # memset can't emit float32r; write the ones column
                        # via a casting DMA (1.0 is exact in any rounding)
                        nc.gpsimd.dma_start(
                            out=v65[t][:, :, D : D + 2],
                            in_=ones_h[:].rearrange("p (h o) -> p h o", o=2),
                        )
                    else:
                        nc.vector.memset(v65[t][:], 1.0)
                    for nh in range(2):
                        ps = mm1.tile([128, 384], FP, tag="mm")
                        for c in range(CCH):
                            nc.tensor.matmul(
                                ps[:],
                                xT[c][:, t * 128 : (t + 1) * 128],
                                wq2[c][:, nh * 384 : (nh + 1) * 384],
                                start=(c == 0),
                                stop=(c == CCH - 1),
                            )
                        nc.any.tensor_copy(
                            v65[t][:, nh * 6 : (nh + 1) * 6, 0:D],
                            ps.rearrange("p (g d) -> p g d", g=6),
                        )

        # ================= phase 2: attention =================
        with (
            tc.tile_pool(name="expT", bufs=2) as exp_pool,
            tc.tile_pool(name="uT", bufs=2) as uT_pool,
            tc.tile_pool(name="sc", bufs=2, space="PSUM") as sc_psum,
            tc.tile_pool(name="u", bufs=2, space="PSUM") as u_psum,
            tc.tile_pool(name="tpu", bufs=2, space="PSUM") as tpu_psum,
        ):
            for p in range(NPAIR):
                eAB = [
                    exp_pool.tile([128, TC, T], MMDT, tag="expT", name="expT") for _ in range(2)
                ]
                for j in range(TC):  # key-token chunks (m)
                    psAB = [
                        sc_psum.tile([128, T], FP, tag="sc", name="sc") for _ in range(2)
                    ]
                    for half in range(2):
                        base = 64 * half
                        for nh in range(2):
                            nc.tensor.matmul(
                                psAB[half][:, nh * 512 : (nh + 1) * 512],
                                qkT[6 + p][
                                    base : base + 64, j * 128 : (j + 1) * 128
                                ],
                                qkT[p][base : base + 64, nh * 512 : (nh + 1) * 512],
                                start=True,
                                stop=True,
                            )
                        nc.scalar.activation(
                            eAB[half][:, j, :], psAB[half][:], Exp, scale=SCALE
                        )
                for half in range(2):
                    h = 2 * p + half
                    e = eAB[half]
                    # U^T[d, n] = sum_m v_aug[m, d] * expT[m, n]; v stationary,
                    # expT moving at N=512 (f32r full rate). Row 64 holds the
                    # softmax denominator via the ones column of v_aug.
                    uT_sb = uT_pool.tile([D + 2, T], FP, tag="uT", name="uT")
                    for nh in range(2):
                        ups = u_psum.tile([D + 2, 512], FP, tag="u", name="u")
                        for j in range(TC):
                            nc.tensor.matmul(
                                ups[:],
                                v65[j][:, h, :],
                                e[:, j, nh * 512 : (nh + 1) * 512],
                                start=(j == 0),
                                stop=(j == TC - 1),
                            )
                        nc.vector.tensor_copy(
                            uT_sb[:, nh * 512 : (nh + 1) * 512], ups[:]
                        )
                    # transpose U^T back to token-major per 128-token chunk,
                    # then normalize with per-partition reciprocal of row 64
                    for i in range(TC):
                        tps = tpu_psum.tile([128, D + 2], FP, tag="tpu", name="tpu")
                        nc.tensor.transpose(
                            tps[:],
                            uT_sb[:, i * 128 : (i + 1) * 128],
                            identity[0 : D + 2, 0 : D + 2],
                        )
                        r = r_pool.tile([128, 1], FP, tag="r")
                        nc.vector.reciprocal(r[:], tps[:, D : D + 1])
                        nc.vector.tensor_scalar_mul(
                            attn_out[i][:, h * D : (h + 1) * D], tps[:, 0:D], r[:]
                        )

        # ================= phase 3: attn_outT, proj =================
        with (
            tc.tile_pool(name="aoT", bufs=CCH) as aoT_pool,
            tc.tile_pool(name="tp2", bufs=2, space="PSUM") as tp2,
            tc.tile_pool(name="pj", bufs=4, space="PSUM") as pj,
        ):
            aoT = [aoT_pool.tile([128, T], MMDT, tag="aoT", name="aoT") for _ in range(CCH)]
            for t in range(TC):
                for c in range(CCH):
                    ps = tp2.tile([128, 128], FP, tag="tp")
                    nc.tensor.transpose(
                        ps[:], attn_out[t][:, c * 128 : (c + 1) * 128], identity[:]
                    )
                    nc.any.tensor_copy(aoT[c][:, t * 128 : (t + 1) * 128], ps[:])
            for t in range(TC):
                y = y_pool.tile([128, C], FP, tag="y")
                for nh in range(2):
                    ps = pj.tile([128, 384], FP, tag="pj")
                    for c in range(CCH):
                        nc.tensor.matmul(
                            ps[:],
                            aoT[c][:, t * 128 : (t + 1) * 128],
                            wp[c][:, nh * 384 : (nh + 1) * 384],
                            start=(c == 0),
                            stop=(c == CCH - 1),
                        )
                    nc.vector.tensor_add(
                        y[:, nh * 384 : (nh + 1) * 384],
                        ps[:],
                        b_bcast[:, nh * 384 : (nh + 1) * 384],
                    )
                nc.sync.dma_start(outa[t * 128 : (t + 1) * 128, :], y[:])

    nc.finalize()
    return nc


_NC_CACHE = {}


def _get_nc():
    if "nc" not in _NC_CACHE:
        import os

        fast = os.environ.get("KERNEL_FAST", "1") == "1"
        _NC_CACHE["nc"] = build(fast=fast)
    return _NC_CACHE["nc"]


def kernel(x, w_qkv, w_proj, b_proj):
    """Full inputs in, full output out. Shards batch across 8 NeuronCores."""
    assert x.shape == (N_CORES, T, C), x.shape
    nc = _get_nc()
    in_maps = [
        {
            "x": np.ascontiguousarray(x[i], dtype=np.float32),
            "w_qkv": np.ascontiguousarray(w_qkv, dtype=np.float32),
            "w_proj": np.ascontiguousarray(w_proj, dtype=np.float32),
            "b_proj": np.ascontiguousarray(b_proj, dtype=np.float32),
        }
        for i in range(N_CORES)
    ]
    res = run_bass_kernel_spmd(nc, in_maps, list(range(N_CORES)))
    return np.stack([res.results[i]["out"] for i in range(N_CORES)], axis=0)


# revision 12
# speedup vs baseline: 1.9219x; 1.0104x over previous
"""Multi-head attention (B=8, N=1024, C=768, H=12) on 8 Trainium2 NeuronCores.

Strategy: pure data parallelism over the batch dimension — each of the 8
cores computes full attention for one batch element; weights are
replicated. No collectives needed.

Per-core dataflow (all matmuls expressed as out = lhsT.T @ rhs on the PE):
  1. xT  = transpose(x)                          (PE identity-transpose, 48 blocks)
  2. qkT = w_qkv[:, :1536].T @ xT  (q,k feature-major)   lhsT = w_qkv chunks
     v   = x @ w_qkv[:, 1536:]    (v token-major)        lhsT = xT chunks
  3. per head pair (2 heads share a 128-row qkT chunk → row-tiled K=64 matmuls):
       scoresT[m,n] = k_h @ q_h^T   (lhsT = kT slice, rhs = qT slice)
       expT = exp(scale * scoresT)  (ScalarE, softmax max-subtraction skipped:
                                     |scores*scale| < ~2, exp is safe in fp32)
       U[n, 0:65] += expT[mchunk].T @ [v_h | 1]  (ones column gives the softmax
                                                  denominator in U[:, 64])
       attn_out[n, h*64:(h+1)*64] = U[:, :64] * (1/U[:, 64])
  4. attn_outT = transpose(attn_out); y = attn_outT.T @ w_proj + b
"""

import os
import sys

for _p in ("/opt/trn_rl_repo", "/root/.axon_site/_ro/trn_rl_repo"):
    if os.path.isdir(_p) and _p not in sys.path:
        sys.path.append(_p)

from contextlib import ExitStack

import numpy as np

import concourse.bass as bass
import concourse.tile as tile
from concourse import bacc, mybir
from concourse.bass_utils import run_bass_kernel_spmd
from concourse.masks import make_identity

FP = mybir.dt.float32
BF16 = mybir.dt.bfloat16
F32R = mybir.dt.float32r
N_CORES = 8
T = 1024  # tokens per core (batch element)
C = 768
H = 12
D = 64
SCALE = D ** (-0.5)
TC = T // 128  # 8 token chunks
CCH = C // 128  # 6 channel chunks
NPAIR = H // 2  # 6 head pairs

Exp = mybir.ActivationFunctionType.Exp


def build(n_cores: int = N_CORES, fast: bool = True):
    # fast=True: run the large matmuls (qkv / scores / proj, all N>=256) with
    # operands tagged float32r — the PE's full-rate fp32 mode (1 cycle/row vs
    # 4 for plain fp32). Storage is IEEE fp32 either way; only the matmul
    # interpretation changes.
    MMDT = F32R if fast else FP
    nc = bacc.Bacc(
        "TRN2", target_bir_lowering=False, debug=False, num_devices=n_cores
    )
    wdma = nc.gpsimd.dma_start if fast else nc.sync.dma_start
    x = nc.declare_dram_parameter("x", [T, C], FP, isOutput=False)
    w_qkv = nc.declare_dram_parameter("w_qkv", [C, 3 * C], FP, isOutput=False)
    w_proj = nc.declare_dram_parameter("w_proj", [C, C], FP, isOutput=False)
    b_proj = nc.declare_dram_parameter("b_proj", [C], FP, isOutput=False)
    out = nc.declare_dram_parameter("out", [T, C], FP, isOutput=True)

    xa, wqa, wpa, outa = x.ap(), w_qkv.ap(), w_proj.ap(), out.ap()
    ba = b_proj.ap()
    b_bcast_src = bass.AP(tensor=ba.tensor, offset=ba.offset, ap=[[0, 128]] + ba.ap)

    with tile.TileContext(nc) as tc, ExitStack() as ctx:
        # ---- persistent pools (live for the whole kernel) ----
        consts = ctx.enter_context(tc.tile_pool(name="consts", bufs=1))
        qk_pool = ctx.enter_context(tc.tile_pool(name="qk", bufs=12))
        v_pool = ctx.enter_context(tc.tile_pool(name="v65", bufs=TC))
        ao_pool = ctx.enter_context(tc.tile_pool(name="attn_out", bufs=TC))
        wp_pool = ctx.enter_context(tc.tile_pool(name="wp", bufs=CCH))
        y_pool = ctx.enter_context(tc.tile_pool(name="y", bufs=2))
        r_pool = ctx.enter_context(tc.tile_pool(name="r", bufs=4))

        identity = consts.tile([128, 128], FP)
        make_identity(nc, identity)
        ones_h = consts.tile([128, 2 * H], FP)
        nc.vector.memset(ones_h[:], 1.0)
        b_bcast = consts.tile([128, C], FP)
        nc.sync.dma_start(b_bcast[:], b_bcast_src)

        wp = []
        for c in range(CCH):
            w = wp_pool.tile([128, C], MMDT, tag="wp")
            wdma(w[:], wpa[c * 128 : (c + 1) * 128, :])
            wp.append(w)

        v65 = [v_pool.tile([128, H, D + 2], BF16 if fast else FP, tag="v65", name="v65") for _ in range(TC)]
        attn_out = [ao_pool.tile([128, C], FP, tag="ao", name="ao") for _ in range(TC)]
        qkT = [qk_pool.tile([128, T], MMDT, tag="qk", name="qk") for _ in range(12)]

        # ================= phase 1: xT, qkT, v =================
        with (
            tc.tile_pool(name="xstage", bufs=2) as xs_pool,
            tc.tile_pool(name="xT", bufs=CCH) as xT_pool,
            tc.tile_pool(name="mm1", bufs=4, space="PSUM") as mm1,
            tc.tile_pool(name="tp1", bufs=2, space="PSUM") as tp1,
        ):
            xT = [xT_pool.tile([128, T], MMDT, tag="xT", name="xT") for _ in range(CCH)]
            for t in range(TC):
                xs = xs_pool.tile([128, C], FP, tag="xs")
                nc.sync.dma_start(xs[:], xa[t * 128 : (t + 1) * 128, :])
                for c in range(CCH):
                    ps = tp1.tile([128, 128], FP, tag="tp")
                    nc.tensor.transpose(
                        ps[:], xs[:, c * 128 : (c + 1) * 128], identity[:]
                    )
                    nc.any.tensor_copy(xT[c][:, t * 128 : (t + 1) * 128], ps[:])

            # family 1: qT/kT feature-major (qkT rows 0:1536)
            with tc.tile_pool(name="wq1", bufs=CCH) as wq1_pool:
                wq1 = []
                for c in range(CCH):
                    w = wq1_pool.tile([128, 2 * C], MMDT, tag="wq1")
                    wdma(w[:], wqa[c * 128 : (c + 1) * 128, 0 : 2 * C])
                    wq1.append(w)
                for j in (0, 6, 1, 7, 2, 8, 3, 9, 4, 10, 5, 11):
                    for nh in range(2):
                        ps = mm1.tile([128, 512], FP, tag="mm")
                        for c in range(CCH):
                            nc.tensor.matmul(
                                ps[:],
                                wq1[c][:, j * 128 : (j + 1) * 128],
                                xT[c][:, nh * 512 : (nh + 1) * 512],
                                start=(c == 0),
                                stop=(c == CCH - 1),
                            )
                        nc.any.tensor_copy(
                            qkT[j][:, nh * 512 : (nh + 1) * 512], ps[:]
                        )

            # family 2: v token-major, interleaved with ones column
            with tc.tile_pool(name="wq2", bufs=CCH) as wq2_pool:
                wq2 = []
                for c in range(CCH):
                    w = wq2_pool.tile([128, C], MMDT, tag="wq2")
                    wdma(
                        w[:], wqa[c * 128 : (c + 1) * 128, 2 * C : 3 * C]
                    )
                    wq2.append(w)
                for t in range(TC):
                    nc.vector.memset(v65[t][:], 1.0)
                    for nh in range(2):
                        ps = mm1.tile([128, 384], FP, tag="mm")
                        for c in range(CCH):
                            nc.tensor.matmul(
                                ps[:],
                                xT[c][:, t * 128 : (t + 1) * 128],
                                wq2[c][:, nh * 384 : (nh + 1) * 384],
                                start=(c == 0),
                                stop=(c == CCH - 1),
                            )
                        nc.any.tensor_copy(
                            v65[t][:, nh * 6 : (nh + 1) * 6, 0:D],
                            ps.rearrange("p (g d) -> p g d", g=6),
                        )

        # ================= phase 2: attention =================
        with (
            tc.tile_pool(name="expT", bufs=2) as exp_pool,
            tc.tile_pool(name="uT", bufs=2) as uT_pool,
            tc.tile_pool(name="sc", bufs=2, space="PSUM") as sc_psum,
            tc.tile_pool(name="u", bufs=2, space="PSUM") as u_psum,
            tc.tile_pool(name="tpu", bufs=2, space="PSUM") as tpu_psum,
        ):
            for p in range(NPAIR):
                eAB = [
                    exp_pool.tile([128, TC, T], BF16 if fast else FP, tag="expT", name="expT") for _ in range(2)
                ]
                for j in range(TC):  # key-token chunks (m)
                    psAB = [
                        sc_psum.tile([128, T], FP, tag="sc", name="sc") for _ in range(2)
                    ]
                    for half in range(2):
                        base = 64 * half
                        for nh in range(2):
                            nc.tensor.matmul(
                                psAB[half][:, nh * 512 : (nh + 1) * 512],
                                qkT[6 + p][
                                    base : base + 64, j * 128 : (j + 1) * 128
                                ],
                                qkT[p][base : base + 64, nh * 512 : (nh + 1) * 512],
                                start=True,
                                stop=True,
                            )
                        nc.scalar.activation(
                            eAB[half][:, j, :], psAB[half][:], Exp, scale=SCALE
                        )
                for half in range(2):
                    h = 2 * p + half
                    e = eAB[half]
                    # U^T[d, n] = sum_m v_aug[m, d] * expT[m, n]; v stationary,
                    # expT moving at N=512 (f32r full rate). Row 64 holds the
                    # softmax denominator via the ones column of v_aug.
                    uT_sb = uT_pool.tile([D + 2, T], FP, tag="uT", name="uT")
                    upsAB = [
                        u_psum.tile([D + 2, 512], FP, tag="u", name="u")
                        for _ in range(2)
                    ]
                    for j in range(TC):  # lhsT (v) reused across both halves
                        for nh in range(2):
                            nc.tensor.matmul(
                                upsAB[nh][:],
                                v65[j][:, h, :],
                                e[:, j, nh * 512 : (nh + 1) * 512],
                                start=(j == 0),
                                stop=(j == TC - 1),
                            )
                    for nh in range(2):
                        nc.vector.tensor_copy(
                            uT_sb[:, nh * 512 : (nh + 1) * 512], upsAB[nh][:]
                        )
                    # transpose U^T back to token-major per 128-token chunk,
                    # then normalize with per-partition reciprocal of row 64
                    for i in range(TC):
                        tps = tpu_psum.tile([128, D + 2], FP, tag="tpu", name="tpu")
                        nc.tensor.transpose(
                            tps[:],
                            uT_sb[:, i * 128 : (i + 1) * 128],
                            identity[0 : D + 2, 0 : D + 2],
                        )
                        r = r_pool.tile([128, 1], FP, tag="r")
                        nc.vector.reciprocal(r[:], tps[:, D : D + 1])
                        nc.vector.tensor_scalar_mul(
                            attn_out[i][:, h * D : (h + 1) * D], tps[:, 0:D], r[:]
                        )

        # ================= phase 3: attn_outT, proj =================
        with (
            tc.tile_pool(name="aoT", bufs=CCH) as aoT_pool,
            tc.tile_pool(name="tp2", bufs=2, space="PSUM") as tp2,
            tc.tile_pool(name="pj", bufs=4, space="PSUM") as pj,
        ):
            aoT = [aoT_pool.tile([128, T], MMDT, tag="aoT", name="aoT") for _ in range(CCH)]
            for t in range(TC):
                for c in range(CCH):
                    ps = tp2.tile([128, 128], FP, tag="tp")
                    nc.tensor.transpose(
                        ps[:], attn_out[t][:, c * 128 : (c + 1) * 128], identity[:]
                    )
                    nc.any.tensor_copy(aoT[c][:, t * 128 : (t + 1) * 128], ps[:])
            for t in range(TC):
                y = y_pool.tile([128, C], FP, tag="y")
                for nh in range(2):
                    ps = pj.tile([128, 384], FP, tag="pj")
                    for c in range(CCH):
                        nc.tensor.matmul(
                            ps[:],
                            aoT[c][:, t * 128 : (t + 1) * 128],
                            wp[c][:, nh * 384 : (nh + 1) * 384],
                            start=(c == 0),
                            stop=(c == CCH - 1),
                        )
                    nc.vector.tensor_add(
                        y[:, nh * 384 : (nh + 1) * 384],
                        ps[:],
                        b_bcast[:, nh * 384 : (nh + 1) * 384],
                    )
                nc.sync.dma_start(outa[t * 128 : (t + 1) * 128, :], y[:])

    nc.finalize()
    return nc


_NC_CACHE = {}


def _get_nc():
    if "nc" not in _NC_CACHE:
        import os

        fast = os.environ.get("KERNEL_FAST", "1") == "1"
        _NC_CACHE["nc"] = build(fast=fast)
    return _NC_CACHE["nc"]


def kernel(x, w_qkv, w_proj, b_proj):
    """Full inputs in, full output out. Shards batch across 8 NeuronCores."""
    assert x.shape == (N_CORES, T, C), x.shape
    nc = _get_nc()
    in_maps = [
        {
            "x": np.ascontiguousarray(x[i], dtype=np.float32),
            "w_qkv": np.ascontiguousarray(w_qkv, dtype=np.float32),
            "w_proj": np.ascontiguousarray(w_proj, dtype=np.float32),
            "b_proj": np.ascontiguousarray(b_proj, dtype=np.float32),
        }
        for i in range(N_CORES)
    ]
    res = run_bass_kernel_spmd(nc, in_maps, list(range(N_CORES)))
    return np.stack([res.results[i]["out"] for i in range(N_CORES)], axis=0)


# revision 14
# speedup vs baseline: 2.0656x; 1.0748x over previous
"""Multi-head attention (B=8, N=1024, C=768, H=12) on 8 Trainium2 NeuronCores.

Strategy: pure data parallelism over the batch dimension — each of the 8
cores computes full attention for one batch element; weights are
replicated. No collectives needed.

Per-core dataflow (all matmuls expressed as out = lhsT.T @ rhs on the PE):
  1. xT  = transpose(x)                          (PE identity-transpose, 48 blocks)
  2. qkT = w_qkv[:, :1536].T @ xT  (q,k feature-major)   lhsT = w_qkv chunks
     v   = x @ w_qkv[:, 1536:]    (v token-major)        lhsT = xT chunks
  3. per head pair (2 heads share a 128-row qkT chunk → row-tiled K=64 matmuls):
       scoresT[m,n] = k_h @ q_h^T   (lhsT = kT slice, rhs = qT slice)
       expT = exp(scale * scoresT)  (ScalarE, softmax max-subtraction skipped:
                                     |scores*scale| < ~2, exp is safe in fp32)
       U[n, 0:65] += expT[mchunk].T @ [v_h | 1]  (ones column gives the softmax
                                                  denominator in U[:, 64])
       attn_out[n, h*64:(h+1)*64] = U[:, :64] * (1/U[:, 64])
  4. attn_outT = transpose(attn_out); y = attn_outT.T @ w_proj + b
"""

import os
import sys

for _p in ("/opt/trn_rl_repo", "/root/.axon_site/_ro/trn_rl_repo"):
    if os.path.isdir(_p) and _p not in sys.path:
        sys.path.append(_p)

from contextlib import ExitStack

import numpy as np

import concourse.bass as bass
import concourse.tile as tile
from concourse import bacc, mybir
from concourse.bass_utils import run_bass_kernel_spmd
from concourse.masks import make_identity

FP = mybir.dt.float32
BF16 = mybir.dt.bfloat16
F32R = mybir.dt.float32r
N_CORES = 8
T = 1024  # tokens per core (batch element)
C = 768
H = 12
D = 64
SCALE = D ** (-0.5)
TC = T // 128  # 8 token chunks
CCH = C // 128  # 6 channel chunks
NPAIR = H // 2  # 6 head pairs

Exp = mybir.ActivationFunctionType.Exp


def build(n_cores: int = N_CORES, fast: bool = True):
    # fast=True: run the large matmuls (qkv / scores / proj, all N>=256) with
    # operands tagged float32r — the PE's full-rate fp32 mode (1 cycle/row vs
    # 4 for plain fp32). Storage is IEEE fp32 either way; only the matmul
    # interpretation changes.
    MMDT = F32R if fast else FP
    nc = bacc.Bacc(
        "TRN2", target_bir_lowering=False, debug=False, num_devices=n_cores
    )
    wdma = nc.gpsimd.dma_start if fast else nc.sync.dma_start
    x = nc.declare_dram_parameter("x", [T, C], FP, isOutput=False)
    w_qkv = nc.declare_dram_parameter("w_qkv", [C, 3 * C], FP, isOutput=False)
    w_proj = nc.declare_dram_parameter("w_proj", [C, C], FP, isOutput=False)
    b_proj = nc.declare_dram_parameter("b_proj", [C], FP, isOutput=False)
    out = nc.declare_dram_parameter("out", [T, C], FP, isOutput=True)

    xa, wqa, wpa, outa = x.ap(), w_qkv.ap(), w_proj.ap(), out.ap()
    ba = b_proj.ap()
    b_bcast_src = bass.AP(tensor=ba.tensor, offset=ba.offset, ap=[[0, 128]] + ba.ap)

    with tile.TileContext(nc) as tc, ExitStack() as ctx:
        # ---- persistent pools (live for the whole kernel) ----
        consts = ctx.enter_context(tc.tile_pool(name="consts", bufs=1))
        qk_pool = ctx.enter_context(tc.tile_pool(name="qk", bufs=12))
        v_pool = ctx.enter_context(tc.tile_pool(name="v65", bufs=TC))
        ao_pool = ctx.enter_context(tc.tile_pool(name="attn_out", bufs=TC))
        wp_pool = ctx.enter_context(tc.tile_pool(name="wp", bufs=CCH))
        y_pool = ctx.enter_context(tc.tile_pool(name="y", bufs=2))
        r_pool = ctx.enter_context(tc.tile_pool(name="r", bufs=4))

        identity = consts.tile([128, 128], FP)
        make_identity(nc, identity)
        ones_h = consts.tile([128, 2 * H], FP)
        nc.vector.memset(ones_h[:], 1.0)

        v65 = [v_pool.tile([128, H, 128], BF16 if fast else FP, tag="v65", name="v65") for _ in range(TC)]
        attn_out = [ao_pool.tile([128, C], FP, tag="ao", name="ao") for _ in range(TC)]
        qkT = [qk_pool.tile([128, T], BF16 if fast else FP, tag="qk", name="qk") for _ in range(12)]

        # ================= phase 1: xT, qkT, v =================
        with (
            tc.tile_pool(name="xstage", bufs=2) as xs_pool,
            tc.tile_pool(name="xT", bufs=CCH) as xT_pool,
            tc.tile_pool(name="mm1", bufs=4, space="PSUM") as mm1,
            tc.tile_pool(name="tp1", bufs=2, space="PSUM") as tp1,
        ):
            xT = [xT_pool.tile([128, T], MMDT, tag="xT", name="xT") for _ in range(CCH)]
            for t in range(TC):
                xs = xs_pool.tile([128, C], FP, tag="xs")
                nc.sync.dma_start(xs[:], xa[t * 128 : (t + 1) * 128, :])
                for c in range(CCH):
                    ps = tp1.tile([128, 128], FP, tag="tp")
                    nc.tensor.transpose(
                        ps[:], xs[:, c * 128 : (c + 1) * 128], identity[:]
                    )
                    nc.any.tensor_copy(xT[c][:, t * 128 : (t + 1) * 128], ps[:])

            # family 1: qT/kT feature-major (qkT rows 0:1536)
            with tc.tile_pool(name="wq1", bufs=CCH) as wq1_pool:
                wq1 = []
                for c in range(CCH):
                    w = wq1_pool.tile([128, 2 * C], MMDT, tag="wq1")
                    wdma(w[:], wqa[c * 128 : (c + 1) * 128, 0 : 2 * C])
                    wq1.append(w)
                for j in (0, 6, 1, 7, 2, 8, 3, 9, 4, 10, 5, 11):
                    for nh in range(2):
                        ps = mm1.tile([128, 512], FP, tag="mm")
                        for c in range(CCH):
                            nc.tensor.matmul(
                                ps[:],
                                wq1[c][:, j * 128 : (j + 1) * 128],
                                xT[c][:, nh * 512 : (nh + 1) * 512],
                                start=(c == 0),
                                stop=(c == CCH - 1),
                            )
                        nc.any.tensor_copy(
                            qkT[j][:, nh * 512 : (nh + 1) * 512], ps[:]
                        )

            # family 2: v token-major, interleaved with ones column
            with tc.tile_pool(name="wq2", bufs=CCH) as wq2_pool:
                wq2 = []
                for c in range(CCH):
                    w = wq2_pool.tile([128, C], MMDT, tag="wq2")
                    wdma(
                        w[:], wqa[c * 128 : (c + 1) * 128, 2 * C : 3 * C]
                    )
                    wq2.append(w)
                for t in range(TC):
                    nc.vector.memset(v65[t][:], 1.0)
                    for nh in range(2):
                        ps = mm1.tile([128, 384], FP, tag="mm")
                        for c in range(CCH):
                            nc.tensor.matmul(
                                ps[:],
                                xT[c][:, t * 128 : (t + 1) * 128],
                                wq2[c][:, nh * 384 : (nh + 1) * 384],
                                start=(c == 0),
                                stop=(c == CCH - 1),
                            )
                        nc.any.tensor_copy(
                            v65[t][:, nh * 6 : (nh + 1) * 6, 0:D],
                            ps.rearrange("p (g d) -> p g d", g=6),
                        )

        # proj weights + bias: only needed in phase 3; issue DMAs after the
        # phase-1 weight loads so they don't starve the first matmuls
        wp = []
        for c in range(CCH):
            w = wp_pool.tile([128, C], MMDT, tag="wp")
            wdma(w[:], wpa[c * 128 : (c + 1) * 128, :])
            wp.append(w)
        b_bcast = consts.tile([128, C], FP)
        nc.sync.dma_start(b_bcast[:], b_bcast_src)

        # ================= phase 2: attention =================
        with (
            tc.tile_pool(name="expT", bufs=2) as exp_pool,
            tc.tile_pool(name="uT", bufs=2) as uT_pool,
            tc.tile_pool(name="sc", bufs=2, space="PSUM") as sc_psum,
            tc.tile_pool(name="u", bufs=2, space="PSUM") as u_psum,
            tc.tile_pool(name="tpu", bufs=2, space="PSUM") as tpu_psum,
        ):
            for p in range(NPAIR):
                eAB = [
                    exp_pool.tile([128, TC, T], BF16 if fast else FP, tag="expT", name="expT") for _ in range(2)
                ]
                for j in range(TC):  # key-token chunks (m)
                    psAB = [
                        sc_psum.tile([128, T], FP, tag="sc", name="sc") for _ in range(2)
                    ]
                    for half in range(2):
                        base = 64 * half
                        for nh in range(2):
                            nc.tensor.matmul(
                                psAB[half][:, nh * 512 : (nh + 1) * 512],
                                qkT[6 + p][
                                    base : base + 64, j * 128 : (j + 1) * 128
                                ],
                                qkT[p][base : base + 64, nh * 512 : (nh + 1) * 512],
                                start=True,
                                stop=True,
                            )
                        nc.scalar.activation(
                            eAB[half][:, j, :], psAB[half][:], Exp, scale=SCALE
                        )
                for half in range(2):
                    h = 2 * p + half
                    e = eAB[half]
                    # U^T[d, n] = sum_m v_aug[m, d] * expT[m, n]; v stationary,
                    # expT moving at N=512 (f32r full rate). Row 64 holds the
                    # softmax denominator via the ones column of v_aug.
                    uT_sb = uT_pool.tile([D + 2, T], FP, tag="uT", name="uT")
                    upsAB = [
                        u_psum.tile([128, 512], FP, tag="u", name="u")
                        for _ in range(2)
                    ]
                    for j in range(TC):  # lhsT (v) reused across both halves
                        for nh in range(2):
                            nc.tensor.matmul(
                                upsAB[nh][:],
                                v65[j][:, h, :],
                                e[:, j, nh * 512 : (nh + 1) * 512],
                                start=(j == 0),
                                stop=(j == TC - 1),
                            )
                    for nh in range(2):
                        nc.vector.tensor_copy(
                            uT_sb[:, nh * 512 : (nh + 1) * 512],
                            upsAB[nh][0 : D + 2, :],
                        )
                    # transpose U^T back to token-major per 128-token chunk,
                    # then normalize with per-partition reciprocal of row 64
                    for i in range(TC):
                        tps = tpu_psum.tile([128, D + 2], FP, tag="tpu", name="tpu")
                        nc.tensor.transpose(
                            tps[:],
                            uT_sb[:, i * 128 : (i + 1) * 128],
                            identity[0 : D + 2, 0 : D + 2],
                        )
                        r = r_pool.tile([128, 1], FP, tag="r")
                        nc.vector.reciprocal(r[:], tps[:, D : D + 1])
                        nc.vector.tensor_scalar_mul(
                            attn_out[i][:, h * D : (h + 1) * D], tps[:, 0:D], r[:]
                        )

        # ================= phase 3: attn_outT, proj =================
        with (
            tc.tile_pool(name="aoT", bufs=CCH) as aoT_pool,
            tc.tile_pool(name="tp2", bufs=2, space="PSUM") as tp2,
            tc.tile_pool(name="pj", bufs=4, space="PSUM") as pj,
        ):
            aoT = [aoT_pool.tile([128, T], MMDT, tag="aoT", name="aoT") for _ in range(CCH)]
            for t in range(TC):
                for c in range(CCH):
                    ps = tp2.tile([128, 128], FP, tag="tp")
                    nc.tensor.transpose(
                        ps[:], attn_out[t][:, c * 128 : (c + 1) * 128], identity[:]
                    )
                    nc.any.tensor_copy(aoT[c][:, t * 128 : (t + 1) * 128], ps[:])
            for t in range(TC):
                y = y_pool.tile([128, C], FP, tag="y")
                for nh in range(2):
                    ps = pj.tile([128, 384], FP, tag="pj")
                    for c in range(CCH):
                        nc.tensor.matmul(
                            ps[:],
                            aoT[c][:, t * 128 : (t + 1) * 128],
                            wp[c][:, nh * 384 : (nh + 1) * 384],
                            start=(c == 0),
                            stop=(c == CCH - 1),
                        )
                    nc.vector.tensor_add(
                        y[:, nh * 384 : (nh + 1) * 384],
                        ps[:],
                        b_bcast[:, nh * 384 : (nh + 1) * 384],
                    )
                nc.sync.dma_start(outa[t * 128 : (t + 1) * 128, :], y[:])

    nc.finalize()
    return nc


_NC_CACHE = {}


def _get_nc():
    if "nc" not in _NC_CACHE:
        import os

        fast = os.environ.get("KERNEL_FAST", "1") == "1"
        _NC_CACHE["nc"] = build(fast=fast)
    return _NC_CACHE["nc"]


def kernel(x, w_qkv, w_proj, b_proj):
    """Full inputs in, full output out. Shards batch across 8 NeuronCores."""
    assert x.shape == (N_CORES, T, C), x.shape
    nc = _get_nc()
    in_maps = [
        {
            "x": np.ascontiguousarray(x[i], dtype=np.float32),
            "w_qkv": np.ascontiguousarray(w_qkv, dtype=np.float32),
            "w_proj": np.ascontiguousarray(w_proj, dtype=np.float32),
            "b_proj": np.ascontiguousarray(b_proj, dtype=np.float32),
        }
        for i in range(N_CORES)
    ]
    res = run_bass_kernel_spmd(nc, in_maps, list(range(N_CORES)))
    return np.stack([res.results[i]["out"] for i in range(N_CORES)], axis=0)
